# revision 12
# baseline (speedup 1.0000x reference)
"""CrissCrossAttention (multi-scale dilated conv + criss-cross axial attention)
Trainium2 Bass/Tile kernel, 8 NeuronCores.

Sharding: 8 cores = 4 batch samples x 2 H-halves. Each core computes the
multi-scale conv (3 dilated 3x3 convs folded into 25 unique sparse taps)
as fp8 DoubleRow matmuls (2x PE throughput; host pre-scales x by 16 and
w by 64, and divides the q/k/v projection weights by 1024 to compensate),
projects q/k/v, exchanges its half of k and v^T with its pair-partner via
AllGather (v split in two w-halves so the exchange and the strided
re-layout DMAs overlap the energy/apply compute), then runs the
criss-cross attention with batched exp / tensor_reduce softmax and
PSUM-batched applies.
"""

import numpy as np
import ml_dtypes

BF16 = ml_dtypes.bfloat16
F8 = ml_dtypes.float8_e4m3

B, C, H, W = 4, 256, 96, 96
CQ = 32
HC = 48            # rows per core
NPOS = HC * W      # 4608 positions per core
HP, WP = HC + 6, W + 6
NCORES = 8
NEG = -1e30
WH = W // 2        # v exchanged in two w-halves

NT_PROJ = 9        # 9 N-tiles of 512
PROJ_N = 512
# conv output tiles: (pos0, n, local row0) -- 9x5 rows + 1x3 rows
CONV_TILES = [(480 * n, 480, 5 * n) for n in range(9)] + [(4320, 288, 45)]
CONV_GROUPS = [CONV_TILES[0:4], CONV_TILES[4:8], CONV_TILES[8:10]]

XS = 16.0          # fp8 input scale
WS = 64.0          # fp8 weight scale  (descale 1/1024 folded into wq/wk/wv)


def _fold_taps(w_ms):
    taps = {}
    for i, d in enumerate((1, 2, 3)):
        for iy in range(3):
            for ix in range(3):
                off = ((iy - 1) * d, (ix - 1) * d)
                if off in taps:
                    taps[off] = taps[off] + w_ms[i][:, :, iy, ix]
                else:
                    taps[off] = w_ms[i][:, :, iy, ix].copy()
    offs = sorted(taps)
    assert len(offs) == 25
    return offs, taps


def _build_program(gamma_f, offs):
    import concourse.mybir as mybir
    import concourse.tile as tile
    from concourse import bacc
    from concourse.bass import ts
    from concourse.masks import make_identity

    dt = mybir.dt
    DR = mybir.MatmulPerfMode.DoubleRow
    nc = bacc.Bacc("TRN2", target_bir_lowering=False, debug=False,
                   num_devices=NCORES)

    xpad_d = nc.dram_tensor("xpad", [128, 2, HP, WP], dt.float8e4, kind="ExternalInput")
    w25_d = nc.dram_tensor("w25", [128, 25, 2, 2, 128], dt.float8e4, kind="ExternalInput")
    wqT_d = nc.dram_tensor("wqT", [2, 128, CQ], dt.bfloat16, kind="ExternalInput")
    wkT_d = nc.dram_tensor("wkT", [2, 128, CQ], dt.bfloat16, kind="ExternalInput")
    wvT_d = nc.dram_tensor("wvT", [2, 128, 256], dt.bfloat16, kind="ExternalInput")
    bq_d = nc.dram_tensor("bq", [CQ, 1], dt.float32, kind="ExternalInput")
    bk_d = nc.dram_tensor("bk", [CQ, 1], dt.float32, kind="ExternalInput")
    bsum_d = nc.dram_tensor("bsum", [2, 128, 1], dt.float32, kind="ExternalInput")
    dmask_d = nc.dram_tensor("dmask", [HC, H], dt.float32, kind="ExternalInput")
    xres_d = nc.dram_tensor("xres", [2, 128, NPOS], dt.float32, kind="ExternalInput")
    out_d = nc.dram_tensor("out", [2, 128, NPOS], dt.float32, kind="ExternalOutput")

    with tile.TileContext(nc) as tc:
        with (
            tc.tile_pool(name="const", bufs=1) as constp,
            tc.tile_pool(name="dram", bufs=1, space="DRAM") as dramp,
            tc.tile_pool(name="accp", bufs=1) as accp,
            tc.tile_pool(name="attp", bufs=1) as attp,
            tc.tile_pool(name="midp", bufs=1) as midp,
            tc.tile_pool(name="smallp", bufs=1) as smallp,
        ):
            # ---- constants ----
            id_bf = constp.tile([128, 128], dt.bfloat16, tag="idbf", name="id_bf")
            make_identity(nc, id_bf)
            id_f32 = constp.tile([96, 96], dt.float32, tag="idf32", name="id_f32")
            make_identity(nc, id_f32)
            bq_sb = constp.tile([CQ, 1], dt.float32, tag="bq", name="bq_sb")
            nc.sync.dma_start(out=bq_sb, in_=bq_d[:])
            bk_sb = constp.tile([CQ, 1], dt.float32, tag="bk", name="bk_sb")
            nc.sync.dma_start(out=bk_sb, in_=bk_d[:])
            bsum_sb = [constp.tile([128, 1], dt.float32, tag=f"bs{m}", name=f"bsum{m}")
                       for m in range(2)]
            for m in range(2):
                nc.sync.dma_start(out=bsum_sb[m], in_=bsum_d[m])
            dmask_sb = constp.tile([HC, H], dt.float32, tag="dm", name="dmask_sb")
            nc.sync.dma_start(out=dmask_sb, in_=dmask_d[:])

            # ---- persistent tensors ----
            acc = [accp.tile([128, NPOS], dt.bfloat16, tag=f"acc{m}", name=f"acc{m}")
                   for m in range(2)]
            attH = attp.tile([HC, W, H], dt.bfloat16, tag="attH", name="attH")
            attW = attp.tile([W, HC, W], dt.bfloat16, tag="attW", name="attW")
            awnT = attp.tile([W, HC, W], dt.bfloat16, tag="awnT", name="awnT")
            ahnT = attp.tile([H, W, HC], dt.bfloat16, tag="ahnT", name="ahnT")
            q_sb = midp.tile([CQ, NPOS], dt.bfloat16, tag="q", name="q_sb")
            k_sb = midp.tile([CQ, NPOS], dt.bfloat16, tag="k", name="k_sb")
            v_whc = midp.tile([W, HC, 256], dt.bfloat16, tag="vwhc", name="v_whc")
            sH = smallp.tile([HC, W], dt.float32, tag="sH", name="sH")
            sW = smallp.tile([W, HC], dt.float32, tag="sWt", name="sW")
            s_h = smallp.tile([HC, W], dt.float32, tag="s_h", name="s_h")
            recip_h = smallp.tile([HC, W], dt.float32, tag="rh", name="recip_h")
            recip_w = smallp.tile([W, HC], dt.float32, tag="rw", name="recip_w")

            # ---- dram bounce buffers for the pair exchange ----
            pack_k = dramp.tile([CQ, NPOS], dt.bfloat16, tag="pk", name="pack_k")
            pack_v = [dramp.tile([WH, HC, 256], dt.bfloat16, tag=f"pv{i}",
                                 name=f"pack_v{i}") for i in range(2)]
            gath_k = dramp.tile([2, CQ, NPOS], dt.bfloat16, tag="gk", name="gath_k")
            gath_v = [dramp.tile([2, WH, HC, 256], dt.bfloat16, tag=f"gv{i}",
                                 name=f"gath_v{i}") for i in range(2)]

            with tc.tile_pool(name="msp", bufs=1) as msp:
                ms_hw = [msp.tile([128, NPOS], dt.bfloat16, tag=f"ms{m}", name=f"ms{m}")
                         for m in range(2)]

                # ============ Phase 1: conv (25 taps, fp8 DoubleRow) ========
                with (
                    tc.tile_pool(name="xw", bufs=1) as xwp,
                    tc.tile_pool(name="cvps", bufs=1, space="PSUM") as cvps,
                ):
                    xpad_sb = xwp.tile([128, 2, HP, WP], dt.float8e4, tag="xp",
                                       name="xpad_sb")
                    nc.sync.dma_start(out=xpad_sb[:, :, 0:26, :],
                                      in_=xpad_d[:, :, 0:26, :])
                    nc.scalar.dma_start(out=xpad_sb[:, :, 26:HP, :],
                                        in_=xpad_d[:, :, 26:HP, :])
                    w25_sb = xwp.tile([128, 25, 2, 2, 128], dt.float8e4, tag="wt",
                                      name="w25_sb")
                    nc.gpsimd.dma_start(out=w25_sb, in_=w25_d[:])

                    for g, gt in enumerate(CONV_GROUPS):
                        P = [[cvps.tile([128, 480], dt.float32, tag=f"cv{m}{j}",
                                        name=f"P{g}{m}{j}", bufs=1)
                              for j in range(len(gt))] for m in range(2)]
                        for t in range(25):
                            dy, dx = offs[t]
                            for m in range(2):
                                lhsT = w25_sb[:, t, m]
                                for j, (p0, n, r0) in enumerate(gt):
                                    rows = n // W
                                    rhs = xpad_sb[:, :, r0 + 3 + dy: r0 + 3 + dy + rows,
                                                  3 + dx: 3 + dx + W]
                                    nc.tensor.matmul(P[m][j][:, 0:n], lhsT, rhs,
                                                     start=(t == 0), stop=(t == 24),
                                                     perf_mode=DR)
                        for m in range(2):
                            for j, (p0, n, r0) in enumerate(gt):
                                nc.vector.tensor_scalar_add(
                                    out=ms_hw[m][:, p0:p0 + n],
                                    in0=P[m][j][:, 0:n], scalar1=bsum_sb[m])

                ms3 = [ms_hw[k].rearrange("p (h w) -> p h w", w=W) for k in range(2)]

                # ======== Phase 2: projections + pair exchange ========
                with (
                    tc.tile_pool(name="pjps", bufs=1, space="PSUM") as pjps,
                    tc.tile_pool(name="pjcp", bufs=1) as pjcp,
                    tc.tile_pool(name="wproj", bufs=1) as wpp,
                ):
                    wqT_sb = [wpp.tile([128, CQ], dt.bfloat16, tag=f"wq{k}",
                                       name=f"wq{k}") for k in range(2)]
                    wkT_sb = [wpp.tile([128, CQ], dt.bfloat16, tag=f"wk{k}",
                                       name=f"wk{k}") for k in range(2)]
                    wvT_sb = [wpp.tile([128, 256], dt.bfloat16, tag=f"wv{k}",
                                       name=f"wv{k}") for k in range(2)]
                    for k in range(2):
                        nc.sync.dma_start(out=wkT_sb[k], in_=wkT_d[k])
                        nc.scalar.dma_start(out=wvT_sb[k], in_=wvT_d[k])
                        nc.sync.dma_start(out=wqT_sb[k], in_=wqT_d[k])

                    # k projection first so its exchange starts early
                    for n in range(NT_PROJ):
                        sl = slice(n * PROJ_N, (n + 1) * PROJ_N)
                        pk = pjps.tile([CQ, PROJ_N], dt.float32, tag="pq",
                                       name=f"pk{n}", bufs=2)
                        for k in range(2):
                            nc.tensor.matmul(pk, wkT_sb[k], ms_hw[k][:, sl],
                                             start=(k == 0), stop=(k == 1))
                        nc.vector.tensor_scalar_add(out=k_sb[:, sl], in0=pk,
                                                    scalar1=bk_sb)
                    nc.sync.dma_start(out=pack_k[:], in_=k_sb[:])
                    nc.gpsimd.collective_compute(
                        "AllGather", mybir.AluOpType.bypass,
                        replica_groups=[[0, 1], [2, 3], [4, 5], [6, 7]],
                        ins=[pack_k[:]], outs=[gath_k[:]])

                    # v^T (own half, w-major chunks) -> pack; two w-halves so
                    # the exchange overlaps the remaining compute
                    def vt_half(i):
                        for w in range(i * WH, (i + 1) * WH):
                            pvt = pjps.tile([HC, 256], dt.float32, tag="pvt",
                                            name=f"pvt{w}", bufs=4)
                            for k in range(2):
                                nc.tensor.matmul(pvt, ms3[k][:, :, w], wvT_sb[k],
                                                 start=(k == 0), stop=(k == 1))
                            stg = pjcp.tile([HC, 256], dt.bfloat16, tag="stg",
                                            name=f"stg{w}", bufs=10)
                            if w % 2 == 0:
                                nc.vector.tensor_copy(out=stg, in_=pvt)
                                nc.sync.dma_start(out=pack_v[i][w - i * WH], in_=stg)
                            else:
                                nc.scalar.copy(out=stg, in_=pvt)
                                nc.scalar.dma_start(out=pack_v[i][w - i * WH], in_=stg)
                        nc.gpsimd.collective_compute(
                            "AllGather", mybir.AluOpType.bypass,
                            replica_groups=[[0, 1], [2, 3], [4, 5], [6, 7]],
                            ins=[pack_v[i][:]], outs=[gath_v[i][:]])

                    vt_half(0)

                    # q projection (between the two v halves: unblocks energies)
                    for n in range(NT_PROJ):
                        sl = slice(n * PROJ_N, (n + 1) * PROJ_N)
                        pq = pjps.tile([CQ, PROJ_N], dt.float32, tag="pq",
                                       name=f"pq{n}", bufs=2)
                        for k in range(2):
                            nc.tensor.matmul(pq, wqT_sb[k], ms_hw[k][:, sl],
                                             start=(k == 0), stop=(k == 1))
                        nc.vector.tensor_scalar_add(out=q_sb[:, sl], in0=pq,
                                                    scalar1=bq_sb)

                    vt_half(1)

                    # own half back from pack_v in w-major layout for row-apply
                    nc.scalar.dma_start(out=v_whc[0:WH], in_=pack_v[0][:])
                    nc.sync.dma_start(out=v_whc[WH:W], in_=pack_v[1][:])
            # msp released here (frees ms before the big attention tensors)

            q3 = q_sb.rearrange("p (h w) -> p h w", w=W)
            k3 = k_sb.rearrange("p (h w) -> p h w", w=W)

            # ================= Phase 3: energies + batched exp =============
            with tc.tile_pool(name="gat", bufs=1) as gatp:
              with (
                tc.tile_pool(name="enps", bufs=1, space="PSUM") as enps,
                tc.tile_pool(name="encp", bufs=1) as encp,
              ):
                  # assemble full-H k and v^T from the gathered halves; the
                  # vT_wo re-layout DMAs are spread over 4 engine queues
                  k_full = gatp.tile([CQ, W, H], dt.bfloat16, tag="kf", name="k_full")
                  for gi in range(2):
                      ko = encp.tile([CQ, NPOS], dt.bfloat16, tag="ko",
                                     name=f"ko{gi}", bufs=1)
                      nc.sync.dma_start(out=ko, in_=gath_k[gi])
                      nc.gpsimd.tensor_copy(
                          out=k_full[:, :, gi * HC:(gi + 1) * HC],
                          in_=ko.rearrange("p (h w) -> p w h", w=W))
                  vT_wo = gatp.tile([H, W, 256], dt.bfloat16, tag="vt", name="vT_wo")
                  qs = [nc.sync, nc.scalar, nc.gpsimd, nc.sync]
                  for i in range(2):
                      for gi in range(2):
                          qs[2 * i + gi].dma_start(
                              out=vT_wo[gi * HC:(gi + 1) * HC,
                                        i * WH:(i + 1) * WH, :],
                              in_=gath_v[i][gi].rearrange("w h c -> h w c"))

                  # row (W) energies: only need own-half q/k; batch 4 h per bank
                  for h0 in range(0, HC, 4):
                      pew = enps.tile([W, 4, W], dt.float32, tag="ew",
                                      name=f"ew{h0}", bufs=3)
                      for j in range(4):
                          nc.tensor.matmul(pew[:, j, :], q3[:, h0 + j, :],
                                           k3[:, h0 + j, :], start=True, stop=True)
                      nc.scalar.activation(out=attW[:, h0:h0 + 4, :], in_=pew,
                                           func=mybir.ActivationFunctionType.Exp)
                  nc.vector.tensor_reduce(out=sW, in_=attW,
                                          axis=mybir.AxisListType.X,
                                          op=mybir.AluOpType.add)

                  # column (H) energies with diagonal mask; batch 4 w per bank;
                  # per-batch sH reduce pipelines with the matmuls
                  for w0 in range(0, W, 4):
                      peh = enps.tile([HC, 4, H], dt.float32, tag="eh",
                                      name=f"eh{w0}", bufs=3)
                      for j in range(4):
                          nc.tensor.matmul(peh[:, j, :], q3[:, :, w0 + j],
                                           k_full[:, w0 + j, :], start=True, stop=True)
                      nc.vector.tensor_add(
                          out=peh, in0=peh,
                          in1=dmask_sb[:, None, :].broadcast_to((HC, 4, H)))
                      nc.scalar.activation(out=attH[:, w0:w0 + 4, :], in_=peh,
                                           func=mybir.ActivationFunctionType.Exp)
                      nc.vector.tensor_reduce(out=sH[:, w0:w0 + 4],
                                              in_=attH[:, w0:w0 + 4, :],
                                              axis=mybir.AxisListType.X,
                                              op=mybir.AluOpType.add)

                  # joint softmax denominators
                  pt1 = enps.tile([HC, W], dt.float32, tag="tr", name="pt1", bufs=1)
                  nc.tensor.transpose(pt1, sW, id_f32)
                  nc.vector.tensor_add(out=s_h, in0=sH, in1=pt1)
                  nc.vector.reciprocal(out=recip_h, in_=s_h)
                  pt2 = enps.tile([W, HC], dt.float32, tag="tr2", name="pt2", bufs=1)
                  nc.tensor.transpose(pt2, recip_h, id_f32[0:HC, 0:HC])
                  nc.vector.tensor_copy(out=recip_w, in_=pt2)

                  # normalize att in place (batched; split vector/gpsimd)
                  for i, h0 in enumerate(range(0, HC, 4)):
                      eng = nc.vector if i % 2 == 0 else nc.gpsimd
                      eng.tensor_mul(
                          out=attW[:, h0:h0 + 4, :], in0=attW[:, h0:h0 + 4, :],
                          in1=recip_w[:, h0:h0 + 4][:, :, None].broadcast_to((W, 4, W)))
                  for i, w0 in enumerate(range(0, W, 4)):
                      eng = nc.vector if i % 2 == 0 else nc.gpsimd
                      eng.tensor_mul(
                          out=attH[:, w0:w0 + 4, :], in0=attH[:, w0:w0 + 4, :],
                          in1=recip_h[:, w0:w0 + 4][:, :, None].broadcast_to((HC, 4, H)))

              # ======== Phase 4a: row attention application ========
              xres0 = gatp.tile([128, NPOS], dt.float32, tag="xr0", name="xres0")
              nc.scalar.dma_start(out=xres0, in_=xres_d[0])
              with (
                  tc.tile_pool(name="apps", bufs=1, space="PSUM") as apps,
              ):
                  # transpose normalized att rows, batched PSUM drain
                  for h0 in range(0, HC, 8):
                      ptw = apps.tile([W, 8, W], dt.bfloat16, tag="tw",
                                      name=f"ptw{h0}", bufs=2)
                      for j in range(8):
                          nc.tensor.transpose(ptw[:, j, :], attW[:, h0 + j, :],
                                              id_bf[0:W, 0:W])
                      nc.vector.tensor_copy(out=awnT[:, h0:h0 + 8, :], in_=ptw)
                  for h0 in range(0, HC, 4):
                      po = [apps.tile([128, 4, W], dt.float32, tag=f"po{m}",
                                      name=f"po{h0}{m}", bufs=2) for m in range(2)]
                      for j in range(4):
                          for m in range(2):
                              nc.tensor.matmul(po[m][:, j, :],
                                               v_whc[:, h0 + j, m * 128:(m + 1) * 128],
                                               awnT[:, h0 + j, :],
                                               start=True, stop=True)
                      nc.vector.tensor_copy(
                          out=acc[0][:, (h0) * W:(h0 + 4) * W], in_=po[0])
                      nc.scalar.copy(
                          out=acc[1][:, (h0) * W:(h0 + 4) * W], in_=po[1])

              # ======== Phase 4b: column attention application ========
              acc3 = [acc[m].rearrange("p (h w) -> p h w", w=W) for m in range(2)]
              with (
                  tc.tile_pool(name="apps2", bufs=1, space="PSUM") as apps2,
              ):
                  for w0 in range(0, W, 8):
                      pth = apps2.tile([H, 8, HC], dt.bfloat16, tag="th",
                                       name=f"pth{w0}", bufs=2)
                      for j in range(8):
                          nc.tensor.transpose(pth[:, j, :], attH[:, w0 + j, :],
                                              id_bf[0:HC, 0:HC])
                      nc.vector.tensor_copy(out=ahnT[:, w0:w0 + 8, :], in_=pth)
                  for w0 in range(0, W, 8):
                      po2 = [apps2.tile([128, 8, HC], dt.float32, tag=f"po2{m}",
                                        name=f"po2{w0}{m}", bufs=2) for m in range(2)]
                      for j in range(8):
                          for m in range(2):
                              nc.tensor.matmul(po2[m][:, j, :],
                                               vT_wo[:, w0 + j, m * 128:(m + 1) * 128],
                                               ahnT[:, w0 + j, :],
                                               start=True, stop=True)
                      for m in range(2):
                          nc.vector.tensor_add(
                              out=acc3[m][:, :, w0:w0 + 8],
                              in0=acc3[m][:, :, w0:w0 + 8],
                              in1=po2[m].rearrange("p w h -> p h w"))

              # ======== Phase 5: residual + output ========
              with tc.tile_pool(name="fin", bufs=1) as finp:
                  for m in range(2):
                      for n in range(NT_PROJ):
                          sl = slice(n * PROJ_N, (n + 1) * PROJ_N)
                          if m == 0:
                              xr = xres0[:, sl]
                          else:
                              xr = finp.tile([128, PROJ_N], dt.float32, tag="xr",
                                             name=f"xr{m}{n}", bufs=4)
                              nc.scalar.dma_start(out=xr, in_=xres_d[m][:, sl])
                          fo = finp.tile([128, PROJ_N], dt.float32, tag="fo",
                                         name=f"fo{m}{n}", bufs=4)
                          nc.vector.scalar_tensor_tensor(
                              out=fo, in0=acc[m][:, sl], scalar=float(gamma_f),
                              in1=xr, op0=mybir.AluOpType.mult,
                              op1=mybir.AluOpType.add)
                          nc.sync.dma_start(out=out_d[m][:, sl], in_=fo)

    nc.compile()
    return nc


def _prepare_inputs(x, w_ms, b_ms, wq, bq, wk, bk, wv, bv, gamma):
    offs, taps = _fold_taps(np.asarray(w_ms, np.float32))
    x = np.asarray(x, np.float32)
    bsum = np.asarray(b_ms, np.float32).sum(0) * (XS * WS)
    gamma_f = float(np.asarray(gamma))
    bv = np.asarray(bv, np.float32)
    inv = 1.0 / (XS * WS)

    # fp8 folded conv weights, laid out [p, tap, m, k_hi, mc]
    w25 = np.empty((128, 25, 2, 2, 128), np.float32)
    for t, off in enumerate(offs):
        wt = taps[off].T.reshape(2, 128, 2, 128)     # [k, p, m, mc]
        w25[:, t] = wt.transpose(1, 2, 0, 3)         # -> [p, m, k, mc]
    w25 = (w25 * WS).astype(F8)
    wqT = (np.asarray(wq, np.float32).T * inv).reshape(2, 128, CQ).astype(BF16)
    wkT = (np.asarray(wk, np.float32).T * inv).reshape(2, 128, CQ).astype(BF16)
    wvT = (np.asarray(wv, np.float32).T * inv).reshape(2, 128, 256).astype(BF16)
    bq_a = np.ascontiguousarray(np.asarray(bq, np.float32).reshape(CQ, 1))
    bk_a = np.ascontiguousarray(np.asarray(bk, np.float32).reshape(CQ, 1))
    bsum_a = np.ascontiguousarray(bsum.reshape(2, 128, 1))

    in_maps = []
    for core in range(NCORES):
        b, g = core // 2, core % 2
        h0 = g * HC
        xp = np.zeros((C, H + 6, W + 6), np.float32)
        xp[:, 3:3 + H, 3:3 + W] = x[b]
        xpad = np.ascontiguousarray(
            (xp[:, h0:h0 + HP, :] * XS).reshape(2, 128, HP, WP)
            .transpose(1, 0, 2, 3)).astype(F8)
        dmask = np.zeros((HC, H), np.float32)
        dmask[np.arange(HC), h0 + np.arange(HC)] = NEG
        xres = (x[b, :, h0:h0 + HC, :].reshape(C, NPOS)
                + gamma_f * bv[:, None]).reshape(2, 128, NPOS)
        in_maps.append({
            "xpad": np.ascontiguousarray(xpad), "w25": w25, "wqT": wqT,
            "wkT": wkT, "wvT": wvT,
            "bq": bq_a, "bk": bk_a, "bsum": bsum_a, "dmask": dmask,
            "xres": np.ascontiguousarray(xres.astype(np.float32)),
        })
    return in_maps, gamma_f, offs


def run(inputs, trace=False):
    from concourse.bass_utils import run_bass_kernel_spmd
    in_maps, gamma_f, offs = _prepare_inputs(**inputs)
    nc = _build_program(gamma_f, offs)
    res = run_bass_kernel_spmd(nc, in_maps, list(range(NCORES)), trace=trace)
    out = np.empty((B, C, H, W), np.float32)
    for core in range(NCORES):
        b, g = core // 2, core % 2
        r = np.asarray(res.results[core]["out"]).reshape(C, HC, W)
        out[b, :, g * HC:(g + 1) * HC, :] = r
    return out, res


def kernel(**inputs) -> np.ndarray:
    out, _ = run(inputs, trace=False)
    return out


# revision 13
# speedup vs baseline: 1.0372x; 1.0372x over previous
"""CrissCrossAttention (multi-scale dilated conv + criss-cross axial attention)
Trainium2 Bass/Tile kernel, 8 NeuronCores.

Sharding: 8 cores = 4 batch samples x 2 H-halves. Each core computes the
multi-scale conv (3 dilated 3x3 convs folded into 25 unique sparse taps)
as fp8 DoubleRow matmuls (2x PE throughput; host pre-scales x by 16 and
w by 64, and divides the q/k/v projection weights by 1024 to compensate),
projects q/k/v, exchanges its half of k and v^T with its pair-partner via
AllGather (v split in two w-halves so the exchange and the strided
re-layout DMAs overlap the energy/apply compute), then runs the
criss-cross attention with batched exp / tensor_reduce softmax and
PSUM-batched applies.
"""

import numpy as np
import ml_dtypes

BF16 = ml_dtypes.bfloat16
F8 = ml_dtypes.float8_e4m3

B, C, H, W = 4, 256, 96, 96
CQ = 32
HC = 48            # rows per core
NPOS = HC * W      # 4608 positions per core
HP, WP = HC + 6, W + 6
NCORES = 8
NEG = -1e30
WH = W // 2        # v exchanged in two w-halves

NT_PROJ = 9        # 9 N-tiles of 512
PROJ_N = 512
# conv output tiles: (pos0, n, local row0) -- 9x5 rows + 1x3 rows
CONV_TILES = [(480 * n, 480, 5 * n) for n in range(9)] + [(4320, 288, 45)]
CONV_GROUPS = [CONV_TILES[0:4], CONV_TILES[4:8], CONV_TILES[8:10]]

XS = 16.0          # fp8 input scale
WS = 64.0          # fp8 weight scale  (descale 1/1024 folded into wq/wk/wv)


def _fold_taps(w_ms):
    taps = {}
    for i, d in enumerate((1, 2, 3)):
        for iy in range(3):
            for ix in range(3):
                off = ((iy - 1) * d, (ix - 1) * d)
                if off in taps:
                    taps[off] = taps[off] + w_ms[i][:, :, iy, ix]
                else:
                    taps[off] = w_ms[i][:, :, iy, ix].copy()
    offs = sorted(taps)
    assert len(offs) == 25
    return offs, taps


def _build_program(gamma_f, offs):
    import concourse.mybir as mybir
    import concourse.tile as tile
    from concourse import bacc
    from concourse.bass import ts
    from concourse.masks import make_identity

    dt = mybir.dt
    DR = mybir.MatmulPerfMode.DoubleRow
    nc = bacc.Bacc("TRN2", target_bir_lowering=False, debug=False,
                   num_devices=NCORES)

    xpad_d = nc.dram_tensor("xpad", [128, 2, HP, WP], dt.float8e4, kind="ExternalInput")
    w25_d = nc.dram_tensor("w25", [128, 25, 2, 2, 128], dt.float8e4, kind="ExternalInput")
    wqT_d = nc.dram_tensor("wqT", [2, 128, CQ], dt.bfloat16, kind="ExternalInput")
    wkT_d = nc.dram_tensor("wkT", [2, 128, CQ], dt.bfloat16, kind="ExternalInput")
    wvT_d = nc.dram_tensor("wvT", [2, 128, 256], dt.bfloat16, kind="ExternalInput")
    bq_d = nc.dram_tensor("bq", [CQ, 1], dt.float32, kind="ExternalInput")
    bk_d = nc.dram_tensor("bk", [CQ, 1], dt.float32, kind="ExternalInput")
    bsum_d = nc.dram_tensor("bsum", [2, 128, 1], dt.float32, kind="ExternalInput")
    dmask_d = nc.dram_tensor("dmask", [HC, H], dt.float32, kind="ExternalInput")
    xres_d = nc.dram_tensor("xres", [2, 128, NPOS], dt.float32, kind="ExternalInput")
    out_d = nc.dram_tensor("out", [2, 128, NPOS], dt.float32, kind="ExternalOutput")

    with tile.TileContext(nc) as tc:
        with (
            tc.tile_pool(name="const", bufs=1) as constp,
            tc.tile_pool(name="dram", bufs=1, space="DRAM") as dramp,
            tc.tile_pool(name="accp", bufs=1) as accp,
            tc.tile_pool(name="attp", bufs=1) as attp,
            tc.tile_pool(name="midp", bufs=1) as midp,
            tc.tile_pool(name="smallp", bufs=1) as smallp,
        ):
            # ---- constants ----
            id_bf = constp.tile([128, 128], dt.bfloat16, tag="idbf", name="id_bf")
            make_identity(nc, id_bf)
            id_f32 = constp.tile([96, 96], dt.float32, tag="idf32", name="id_f32")
            make_identity(nc, id_f32)
            bq_sb = constp.tile([CQ, 1], dt.float32, tag="bq", name="bq_sb")
            nc.sync.dma_start(out=bq_sb, in_=bq_d[:])
            bk_sb = constp.tile([CQ, 1], dt.float32, tag="bk", name="bk_sb")
            nc.sync.dma_start(out=bk_sb, in_=bk_d[:])
            bsum_sb = [constp.tile([128, 1], dt.float32, tag=f"bs{m}", name=f"bsum{m}")
                       for m in range(2)]
            for m in range(2):
                nc.sync.dma_start(out=bsum_sb[m], in_=bsum_d[m])
            dmask_sb = constp.tile([HC, H], dt.float32, tag="dm", name="dmask_sb")
            nc.sync.dma_start(out=dmask_sb, in_=dmask_d[:])

            # ---- persistent tensors ----
            acc = [accp.tile([128, NPOS], dt.bfloat16, tag=f"acc{m}", name=f"acc{m}")
                   for m in range(2)]
            attH = attp.tile([HC, W, H], dt.bfloat16, tag="attH", name="attH")
            attW = attp.tile([W, HC, W], dt.bfloat16, tag="attW", name="attW")
            awnT = attp.tile([W, HC, W], dt.bfloat16, tag="awnT", name="awnT")
            ahnT = attp.tile([H, W, HC], dt.bfloat16, tag="ahnT", name="ahnT")
            q_sb = midp.tile([CQ, NPOS], dt.bfloat16, tag="q", name="q_sb")
            k_sb = midp.tile([CQ, NPOS], dt.bfloat16, tag="k", name="k_sb")
            v_whc = midp.tile([W, HC, 256], dt.bfloat16, tag="vwhc", name="v_whc")
            sH = smallp.tile([HC, W], dt.float32, tag="sH", name="sH")
            sW = smallp.tile([W, HC], dt.float32, tag="sWt", name="sW")
            s_h = smallp.tile([HC, W], dt.float32, tag="s_h", name="s_h")
            recip_h = smallp.tile([HC, W], dt.float32, tag="rh", name="recip_h")
            recip_w = smallp.tile([W, HC], dt.float32, tag="rw", name="recip_w")

            # ---- dram bounce buffers for the pair exchange ----
            pack_k = dramp.tile([CQ, NPOS], dt.bfloat16, tag="pk", name="pack_k")
            pack_v = [dramp.tile([WH, HC, 256], dt.bfloat16, tag=f"pv{i}",
                                 name=f"pack_v{i}") for i in range(2)]
            gath_k = dramp.tile([2, CQ, NPOS], dt.bfloat16, tag="gk", name="gath_k")
            gath_v = [dramp.tile([2, WH, HC, 256], dt.bfloat16, tag=f"gv{i}",
                                 name=f"gath_v{i}") for i in range(2)]

            with tc.tile_pool(name="msp", bufs=1) as msp:
                ms_hw = [msp.tile([128, NPOS], dt.bfloat16, tag=f"ms{m}", name=f"ms{m}")
                         for m in range(2)]

                # ============ Phase 1: conv (25 taps, fp8 DoubleRow) ========
                with (
                    tc.tile_pool(name="xw", bufs=1) as xwp,
                    tc.tile_pool(name="cvps", bufs=1, space="PSUM") as cvps,
                ):
                    xpad_sb = xwp.tile([128, 2, HP, WP], dt.float8e4, tag="xp",
                                       name="xpad_sb")
                    nc.sync.dma_start(out=xpad_sb[:, :, 0:26, :],
                                      in_=xpad_d[:, :, 0:26, :])
                    nc.scalar.dma_start(out=xpad_sb[:, :, 26:HP, :],
                                        in_=xpad_d[:, :, 26:HP, :])
                    w25_sb = xwp.tile([128, 25, 2, 2, 128], dt.float8e4, tag="wt",
                                      name="w25_sb")
                    nc.gpsimd.dma_start(out=w25_sb, in_=w25_d[:])

                    for g, gt in enumerate(CONV_GROUPS):
                        P = [[cvps.tile([128, 480], dt.float32, tag=f"cv{m}{j}",
                                        name=f"P{g}{m}{j}", bufs=1)
                              for j in range(len(gt))] for m in range(2)]
                        for t in range(25):
                            dy, dx = offs[t]
                            for m in range(2):
                                lhsT = w25_sb[:, t, m]
                                for j, (p0, n, r0) in enumerate(gt):
                                    rows = n // W
                                    rhs = xpad_sb[:, :, r0 + 3 + dy: r0 + 3 + dy + rows,
                                                  3 + dx: 3 + dx + W]
                                    nc.tensor.matmul(P[m][j][:, 0:n], lhsT, rhs,
                                                     start=(t == 0), stop=(t == 24),
                                                     perf_mode=DR)
                        for m in range(2):
                            for j, (p0, n, r0) in enumerate(gt):
                                nc.vector.tensor_scalar_add(
                                    out=ms_hw[m][:, p0:p0 + n],
                                    in0=P[m][j][:, 0:n], scalar1=bsum_sb[m])

                ms3 = [ms_hw[k].rearrange("p (h w) -> p h w", w=W) for k in range(2)]

                # ======== Phase 2: projections + pair exchange ========
                with (
                    tc.tile_pool(name="pjps", bufs=1, space="PSUM") as pjps,
                    tc.tile_pool(name="pjcp", bufs=1) as pjcp,
                    tc.tile_pool(name="wproj", bufs=1) as wpp,
                ):
                    wqT_sb = [wpp.tile([128, CQ], dt.bfloat16, tag=f"wq{k}",
                                       name=f"wq{k}") for k in range(2)]
                    wkT_sb = [wpp.tile([128, CQ], dt.bfloat16, tag=f"wk{k}",
                                       name=f"wk{k}") for k in range(2)]
                    wvT_sb = [wpp.tile([128, 256], dt.bfloat16, tag=f"wv{k}",
                                       name=f"wv{k}") for k in range(2)]
                    for k in range(2):
                        nc.sync.dma_start(out=wkT_sb[k], in_=wkT_d[k])
                        nc.scalar.dma_start(out=wvT_sb[k], in_=wvT_d[k])
                        nc.sync.dma_start(out=wqT_sb[k], in_=wqT_d[k])

                    # k projection first so its exchange starts early
                    for n in range(NT_PROJ):
                        sl = slice(n * PROJ_N, (n + 1) * PROJ_N)
                        pk = pjps.tile([CQ, PROJ_N], dt.float32, tag="pq",
                                       name=f"pk{n}", bufs=2)
                        for k in range(2):
                            nc.tensor.matmul(pk, wkT_sb[k], ms_hw[k][:, sl],
                                             start=(k == 0), stop=(k == 1))
                        nc.vector.tensor_scalar_add(out=k_sb[:, sl], in0=pk,
                                                    scalar1=bk_sb)
                    nc.sync.dma_start(out=pack_k[:], in_=k_sb[:])
                    nc.gpsimd.collective_compute(
                        "AllGather", mybir.AluOpType.bypass,
                        replica_groups=[[0, 1], [2, 3], [4, 5], [6, 7]],
                        ins=[pack_k[:]], outs=[gath_k[:]])

                    # v^T (own half, w-major chunks) -> pack; two w-halves so
                    # the exchange overlaps the remaining compute
                    def vt_half(i):
                        for w in range(i * WH, (i + 1) * WH):
                            pvt = pjps.tile([HC, 256], dt.float32, tag="pvt",
                                            name=f"pvt{w}", bufs=4)
                            for k in range(2):
                                nc.tensor.matmul(pvt, ms3[k][:, :, w], wvT_sb[k],
                                                 start=(k == 0), stop=(k == 1))
                            stg = pjcp.tile([HC, 256], dt.bfloat16, tag="stg",
                                            name=f"stg{w}", bufs=10)
                            if w % 2 == 0:
                                nc.vector.tensor_copy(out=stg, in_=pvt)
                                nc.sync.dma_start(out=pack_v[i][w - i * WH], in_=stg)
                            else:
                                nc.scalar.copy(out=stg, in_=pvt)
                                nc.scalar.dma_start(out=pack_v[i][w - i * WH], in_=stg)
                        nc.gpsimd.collective_compute(
                            "AllGather", mybir.AluOpType.bypass,
                            replica_groups=[[0, 1], [2, 3], [4, 5], [6, 7]],
                            ins=[pack_v[i][:]], outs=[gath_v[i][:]])

                    vt_half(0)

                    # q projection (between the two v halves: unblocks energies)
                    for n in range(NT_PROJ):
                        sl = slice(n * PROJ_N, (n + 1) * PROJ_N)
                        pq = pjps.tile([CQ, PROJ_N], dt.float32, tag="pq",
                                       name=f"pq{n}", bufs=2)
                        for k in range(2):
                            nc.tensor.matmul(pq, wqT_sb[k], ms_hw[k][:, sl],
                                             start=(k == 0), stop=(k == 1))
                        nc.vector.tensor_scalar_add(out=q_sb[:, sl], in0=pq,
                                                    scalar1=bq_sb)

                    vt_half(1)

                    # own half back from pack_v in w-major layout for row-apply
                    nc.scalar.dma_start(out=v_whc[0:WH], in_=pack_v[0][:])
                    nc.sync.dma_start(out=v_whc[WH:W], in_=pack_v[1][:])
            # msp released here (frees ms before the big attention tensors)

            q3 = q_sb.rearrange("p (h w) -> p h w", w=W)
            k3 = k_sb.rearrange("p (h w) -> p h w", w=W)

            # ================= Phase 3: energies + batched exp =============
            with tc.tile_pool(name="gat", bufs=1) as gatp:
              with (
                tc.tile_pool(name="enps", bufs=1, space="PSUM") as enps,
                tc.tile_pool(name="encp", bufs=1) as encp,
              ):
                  # assemble full-H k and v^T from the gathered halves; the
                  # vT_wo re-layout DMAs are spread over 4 engine queues
                  k_full = gatp.tile([CQ, W, H], dt.bfloat16, tag="kf", name="k_full")
                  for gi in range(2):
                      ko = encp.tile([CQ, NPOS], dt.bfloat16, tag="ko",
                                     name=f"ko{gi}", bufs=1)
                      nc.sync.dma_start(out=ko, in_=gath_k[gi])
                      nc.vector.tensor_copy(
                          out=k_full[:, :, gi * HC:(gi + 1) * HC],
                          in_=ko.rearrange("p (h w) -> p w h", w=W))
                  vT_wo = gatp.tile([H, W, 256], dt.bfloat16, tag="vt", name="vT_wo")
                  qs = [nc.sync, nc.scalar, nc.gpsimd, nc.sync]
                  for i in range(2):
                      for gi in range(2):
                          qs[2 * i + gi].dma_start(
                              out=vT_wo[gi * HC:(gi + 1) * HC,
                                        i * WH:(i + 1) * WH, :],
                              in_=gath_v[i][gi].rearrange("w h c -> h w c"))

                  # row (W) energies: only need own-half q/k; batch 4 h per bank
                  for h0 in range(0, HC, 4):
                      pew = enps.tile([W, 4, W], dt.float32, tag="ew",
                                      name=f"ew{h0}", bufs=3)
                      for j in range(4):
                          nc.tensor.matmul(pew[:, j, :], q3[:, h0 + j, :],
                                           k3[:, h0 + j, :], start=True, stop=True)
                      nc.scalar.activation(out=attW[:, h0:h0 + 4, :], in_=pew,
                                           func=mybir.ActivationFunctionType.Exp)
                  nc.vector.tensor_reduce(out=sW, in_=attW,
                                          axis=mybir.AxisListType.X,
                                          op=mybir.AluOpType.add)

                  # column (H) energies with diagonal mask; batch 4 w per bank;
                  # per-batch sH reduce pipelines with the matmuls
                  for w0 in range(0, W, 4):
                      peh = enps.tile([HC, 4, H], dt.float32, tag="eh",
                                      name=f"eh{w0}", bufs=3)
                      for j in range(4):
                          nc.tensor.matmul(peh[:, j, :], q3[:, :, w0 + j],
                                           k_full[:, w0 + j, :], start=True, stop=True)
                      nc.vector.tensor_add(
                          out=peh, in0=peh,
                          in1=dmask_sb[:, None, :].broadcast_to((HC, 4, H)))
                      nc.scalar.activation(out=attH[:, w0:w0 + 4, :], in_=peh,
                                           func=mybir.ActivationFunctionType.Exp)
                      nc.vector.tensor_reduce(out=sH[:, w0:w0 + 4],
                                              in_=attH[:, w0:w0 + 4, :],
                                              axis=mybir.AxisListType.X,
                                              op=mybir.AluOpType.add)

                  # joint softmax denominators
                  pt1 = enps.tile([HC, W], dt.float32, tag="tr", name="pt1", bufs=1)
                  nc.tensor.transpose(pt1, sW, id_f32)
                  nc.vector.tensor_add(out=s_h, in0=sH, in1=pt1)
                  nc.vector.reciprocal(out=recip_h, in_=s_h)
                  pt2 = enps.tile([W, HC], dt.float32, tag="tr2", name="pt2", bufs=1)
                  nc.tensor.transpose(pt2, recip_h, id_f32[0:HC, 0:HC])
                  nc.vector.tensor_copy(out=recip_w, in_=pt2)

                  # normalize att in place (batched; split vector/gpsimd)
                  for i, h0 in enumerate(range(0, HC, 4)):
                      eng = nc.vector if i % 2 == 0 else nc.gpsimd
                      eng.tensor_mul(
                          out=attW[:, h0:h0 + 4, :], in0=attW[:, h0:h0 + 4, :],
                          in1=recip_w[:, h0:h0 + 4][:, :, None].broadcast_to((W, 4, W)))
                  for i, w0 in enumerate(range(0, W, 4)):
                      eng = nc.vector if i % 2 == 0 else nc.gpsimd
                      eng.tensor_mul(
                          out=attH[:, w0:w0 + 4, :], in0=attH[:, w0:w0 + 4, :],
                          in1=recip_h[:, w0:w0 + 4][:, :, None].broadcast_to((HC, 4, H)))

              # ======== Phase 4a: row attention application ========
              xres0 = gatp.tile([128, NPOS], dt.float32, tag="xr0", name="xres0")
              nc.scalar.dma_start(out=xres0, in_=xres_d[0])
              with (
                  tc.tile_pool(name="apps", bufs=1, space="PSUM") as apps,
              ):
                  # transpose normalized att rows, batched PSUM drain
                  for h0 in range(0, HC, 8):
                      ptw = apps.tile([W, 8, W], dt.bfloat16, tag="tw",
                                      name=f"ptw{h0}", bufs=2)
                      for j in range(8):
                          nc.tensor.transpose(ptw[:, j, :], attW[:, h0 + j, :],
                                              id_bf[0:W, 0:W])
                      nc.vector.tensor_copy(out=awnT[:, h0:h0 + 8, :], in_=ptw)
                  for h0 in range(0, HC, 4):
                      po = [apps.tile([128, 4, W], dt.float32, tag=f"po{m}",
                                      name=f"po{h0}{m}", bufs=2) for m in range(2)]
                      for j in range(4):
                          for m in range(2):
                              nc.tensor.matmul(po[m][:, j, :],
                                               v_whc[:, h0 + j, m * 128:(m + 1) * 128],
                                               awnT[:, h0 + j, :],
                                               start=True, stop=True)
                      nc.vector.tensor_copy(
                          out=acc[0][:, (h0) * W:(h0 + 4) * W], in_=po[0])
                      nc.scalar.copy(
                          out=acc[1][:, (h0) * W:(h0 + 4) * W], in_=po[1])

              # ======== Phase 4b: column attention application ========
              acc3 = [acc[m].rearrange("p (h w) -> p h w", w=W) for m in range(2)]
              with (
                  tc.tile_pool(name="apps2", bufs=1, space="PSUM") as apps2,
              ):
                  for w0 in range(0, W, 8):
                      pth = apps2.tile([H, 8, HC], dt.bfloat16, tag="th",
                                       name=f"pth{w0}", bufs=2)
                      for j in range(8):
                          nc.tensor.transpose(pth[:, j, :], attH[:, w0 + j, :],
                                              id_bf[0:HC, 0:HC])
                      nc.vector.tensor_copy(out=ahnT[:, w0:w0 + 8, :], in_=pth)
                  for w0 in range(0, W, 8):
                      po2 = [apps2.tile([128, 8, HC], dt.float32, tag=f"po2{m}",
                                        name=f"po2{w0}{m}", bufs=2) for m in range(2)]
                      for j in range(8):
                          for m in range(2):
                              nc.tensor.matmul(po2[m][:, j, :],
                                               vT_wo[:, w0 + j, m * 128:(m + 1) * 128],
                                               ahnT[:, w0 + j, :],
                                               start=True, stop=True)
                      for m in range(2):
                          nc.vector.tensor_add(
                              out=acc3[m][:, :, w0:w0 + 8],
                              in0=acc3[m][:, :, w0:w0 + 8],
                              in1=po2[m].rearrange("p w h -> p h w"))

              # ======== Phase 5: residual + output ========
              with tc.tile_pool(name="fin", bufs=1) as finp:
                  for m in range(2):
                      for n in range(NT_PROJ):
                          sl = slice(n * PROJ_N, (n + 1) * PROJ_N)
                          if m == 0:
                              xr = xres0[:, sl]
                          else:
                              xr = finp.tile([128, PROJ_N], dt.float32, tag="xr",
                                             name=f"xr{m}{n}", bufs=4)
                              nc.scalar.dma_start(out=xr, in_=xres_d[m][:, sl])
                          fo = finp.tile([128, PROJ_N], dt.float32, tag="fo",
                                         name=f"fo{m}{n}", bufs=4)
                          nc.vector.scalar_tensor_tensor(
                              out=fo, in0=acc[m][:, sl], scalar=float(gamma_f),
                              in1=xr, op0=mybir.AluOpType.mult,
                              op1=mybir.AluOpType.add)
                          nc.sync.dma_start(out=out_d[m][:, sl], in_=fo)

    nc.compile()
    return nc


def _prepare_inputs(x, w_ms, b_ms, wq, bq, wk, bk, wv, bv, gamma):
    offs, taps = _fold_taps(np.asarray(w_ms, np.float32))
    x = np.asarray(x, np.float32)
    bsum = np.asarray(b_ms, np.float32).sum(0) * (XS * WS)
    gamma_f = float(np.asarray(gamma))
    bv = np.asarray(bv, np.float32)
    inv = 1.0 / (XS * WS)

    # fp8 folded conv weights, laid out [p, tap, m, k_hi, mc]
    w25 = np.empty((128, 25, 2, 2, 128), np.float32)
    for t, off in enumerate(offs):
        wt = taps[off].T.reshape(2, 128, 2, 128)     # [k, p, m, mc]
        w25[:, t] = wt.transpose(1, 2, 0, 3)         # -> [p, m, k, mc]
    w25 = (w25 * WS).astype(F8)
    wqT = (np.asarray(wq, np.float32).T * inv).reshape(2, 128, CQ).astype(BF16)
    wkT = (np.asarray(wk, np.float32).T * inv).reshape(2, 128, CQ).astype(BF16)
    wvT = (np.asarray(wv, np.float32).T * inv).reshape(2, 128, 256).astype(BF16)
    bq_a = np.ascontiguousarray(np.asarray(bq, np.float32).reshape(CQ, 1))
    bk_a = np.ascontiguousarray(np.asarray(bk, np.float32).reshape(CQ, 1))
    bsum_a = np.ascontiguousarray(bsum.reshape(2, 128, 1))

    in_maps = []
    for core in range(NCORES):
        b, g = core // 2, core % 2
        h0 = g * HC
        xp = np.zeros((C, H + 6, W + 6), np.float32)
        xp[:, 3:3 + H, 3:3 + W] = x[b]
        xpad = np.ascontiguousarray(
            (xp[:, h0:h0 + HP, :] * XS).reshape(2, 128, HP, WP)
            .transpose(1, 0, 2, 3)).astype(F8)
        dmask = np.zeros((HC, H), np.float32)
        dmask[np.arange(HC), h0 + np.arange(HC)] = NEG
        xres = (x[b, :, h0:h0 + HC, :].reshape(C, NPOS)
                + gamma_f * bv[:, None]).reshape(2, 128, NPOS)
        in_maps.append({
            "xpad": np.ascontiguousarray(xpad), "w25": w25, "wqT": wqT,
            "wkT": wkT, "wvT": wvT,
            "bq": bq_a, "bk": bk_a, "bsum": bsum_a, "dmask": dmask,
            "xres": np.ascontiguousarray(xres.astype(np.float32)),
        })
    return in_maps, gamma_f, offs


def run(inputs, trace=False):
    from concourse.bass_utils import run_bass_kernel_spmd
    in_maps, gamma_f, offs = _prepare_inputs(**inputs)
    nc = _build_program(gamma_f, offs)
    res = run_bass_kernel_spmd(nc, in_maps, list(range(NCORES)), trace=trace)
    out = np.empty((B, C, H, W), np.float32)
    for core in range(NCORES):
        b, g = core // 2, core % 2
        r = np.asarray(res.results[core]["out"]).reshape(C, HC, W)
        out[b, :, g * HC:(g + 1) * HC, :] = r
    return out, res


def kernel(**inputs) -> np.ndarray:
    out, _ = run(inputs, trace=False)
    return out


# revision 15
# speedup vs baseline: 1.2396x; 1.1952x over previous
"""CrissCrossAttention (multi-scale dilated conv + criss-cross axial attention)
Trainium2 Bass/Tile kernel, 8 NeuronCores.

Sharding: 8 cores = 4 batch samples x 2 H-halves. Each core computes the
multi-scale conv (3 dilated 3x3 convs folded into 25 unique sparse taps)
as fp8 DoubleRow matmuls (2x PE throughput; host pre-scales x by 16 and
w by 64, and divides the q/k/v projection weights by 1024 to compensate),
projects q/k/v, exchanges its half of k and v^T with its pair-partner via
AllGather (v split in two w-halves so the exchange and the strided
re-layout DMAs overlap the energy/apply compute), then runs the
criss-cross attention with batched exp / tensor_reduce softmax and
PSUM-batched applies.
"""

import numpy as np
import ml_dtypes

BF16 = ml_dtypes.bfloat16
F8 = ml_dtypes.float8_e4m3

B, C, H, W = 4, 256, 96, 96
CQ = 32
HC = 48            # rows per core
NPOS = HC * W      # 4608 positions per core
HP, WP = HC + 6, W + 6
NCORES = 8
NEG = -1e30
WH = W // 2        # v exchanged in two w-halves

NT_PROJ = 9        # 9 N-tiles of 512
PROJ_N = 512
# conv output tiles: (pos0, n, local row0) -- 9x5 rows + 1x3 rows
CONV_TILES = [(480 * n, 480, 5 * n) for n in range(9)] + [(4320, 288, 45)]
CONV_GROUPS = [CONV_TILES[0:4], CONV_TILES[4:8], CONV_TILES[8:10]]

XS = 16.0          # fp8 input scale
WS = 64.0          # fp8 weight scale  (descale 1/1024 folded into wq/wk/wv)


def _fold_taps(w_ms):
    taps = {}
    for i, d in enumerate((1, 2, 3)):
        for iy in range(3):
            for ix in range(3):
                off = ((iy - 1) * d, (ix - 1) * d)
                if off in taps:
                    taps[off] = taps[off] + w_ms[i][:, :, iy, ix]
                else:
                    taps[off] = w_ms[i][:, :, iy, ix].copy()
    offs = sorted(taps)
    assert len(offs) == 25
    return offs, taps


def _build_program(gamma_f, offs):
    import concourse.mybir as mybir
    import concourse.tile as tile
    from concourse import bacc
    from concourse.bass import ts
    from concourse.masks import make_identity

    dt = mybir.dt
    DR = mybir.MatmulPerfMode.DoubleRow
    nc = bacc.Bacc("TRN2", target_bir_lowering=False, debug=False,
                   num_devices=NCORES)

    xpad_d = nc.dram_tensor("xpad", [128, 2, HP, WP], dt.float8e4, kind="ExternalInput")
    w25_d = nc.dram_tensor("w25", [128, 25, 2, 2, 128], dt.float8e4, kind="ExternalInput")
    wqT_d = nc.dram_tensor("wqT", [2, 128, CQ], dt.bfloat16, kind="ExternalInput")
    wkT_d = nc.dram_tensor("wkT", [2, 128, CQ], dt.bfloat16, kind="ExternalInput")
    wvT_d = nc.dram_tensor("wvT", [2, 128, 256], dt.bfloat16, kind="ExternalInput")
    bq_d = nc.dram_tensor("bq", [CQ, 1], dt.float32, kind="ExternalInput")
    bk_d = nc.dram_tensor("bk", [CQ, 1], dt.float32, kind="ExternalInput")
    bsum_d = nc.dram_tensor("bsum", [2, 128, 1], dt.float32, kind="ExternalInput")
    dmask_d = nc.dram_tensor("dmask", [HC, H], dt.float32, kind="ExternalInput")
    xres_d = nc.dram_tensor("xres", [2, 128, NPOS], dt.float32, kind="ExternalInput")
    out_d = nc.dram_tensor("out", [2, 128, NPOS], dt.float32, kind="ExternalOutput")

    with tile.TileContext(nc) as tc:
        with (
            tc.tile_pool(name="const", bufs=1) as constp,
            tc.tile_pool(name="dram", bufs=1, space="DRAM") as dramp,
            tc.tile_pool(name="accp", bufs=1) as accp,
            tc.tile_pool(name="attp", bufs=1) as attp,
            tc.tile_pool(name="midp", bufs=1) as midp,
            tc.tile_pool(name="smallp", bufs=1) as smallp,
        ):
            # ---- constants ----
            id_bf = constp.tile([128, 128], dt.bfloat16, tag="idbf", name="id_bf")
            make_identity(nc, id_bf)
            id_f32 = constp.tile([96, 96], dt.float32, tag="idf32", name="id_f32")
            make_identity(nc, id_f32)
            bq_sb = constp.tile([CQ, 1], dt.float32, tag="bq", name="bq_sb")
            nc.sync.dma_start(out=bq_sb, in_=bq_d[:])
            bk_sb = constp.tile([CQ, 1], dt.float32, tag="bk", name="bk_sb")
            nc.sync.dma_start(out=bk_sb, in_=bk_d[:])
            bsum_sb = [constp.tile([128, 1], dt.float32, tag=f"bs{m}", name=f"bsum{m}")
                       for m in range(2)]
            for m in range(2):
                nc.sync.dma_start(out=bsum_sb[m], in_=bsum_d[m])
            dmask_sb = constp.tile([HC, H], dt.float32, tag="dm", name="dmask_sb")
            nc.sync.dma_start(out=dmask_sb, in_=dmask_d[:])

            # ---- persistent tensors ----
            acc = [accp.tile([128, NPOS], dt.bfloat16, tag=f"acc{m}", name=f"acc{m}")
                   for m in range(2)]
            attH = attp.tile([HC, W, H], dt.bfloat16, tag="attH", name="attH")
            attW = attp.tile([W, HC, W], dt.bfloat16, tag="attW", name="attW")
            awnT = attp.tile([W, HC, W], dt.bfloat16, tag="awnT", name="awnT")
            ahnT = attp.tile([H, W, HC], dt.bfloat16, tag="ahnT", name="ahnT")
            q_sb = midp.tile([CQ, NPOS], dt.bfloat16, tag="q", name="q_sb")
            k_sb = midp.tile([CQ, NPOS], dt.bfloat16, tag="k", name="k_sb")
            v_whc = midp.tile([W, HC, 256], dt.bfloat16, tag="vwhc", name="v_whc")
            sH = smallp.tile([HC, W], dt.float32, tag="sH", name="sH")
            sW = smallp.tile([W, HC], dt.float32, tag="sWt", name="sW")
            s_h = smallp.tile([HC, W], dt.float32, tag="s_h", name="s_h")
            recip_h = smallp.tile([HC, W], dt.float32, tag="rh", name="recip_h")
            recip_w = smallp.tile([W, HC], dt.float32, tag="rw", name="recip_w")

            # ---- dram bounce buffers for the pair exchange ----
            pack_k = dramp.tile([CQ, NPOS], dt.bfloat16, tag="pk", name="pack_k")
            pack_v = [dramp.tile([WH, HC, 256], dt.bfloat16, tag=f"pv{i}",
                                 name=f"pack_v{i}") for i in range(2)]
            gath_k = dramp.tile([2, CQ, NPOS], dt.bfloat16, tag="gk", name="gath_k")
            gath_v = [dramp.tile([2, WH, HC, 256], dt.bfloat16, tag=f"gv{i}",
                                 name=f"gath_v{i}") for i in range(2)]

            with tc.tile_pool(name="msp", bufs=1) as msp:
                ms_hw = [msp.tile([128, NPOS], dt.bfloat16, tag=f"ms{m}", name=f"ms{m}")
                         for m in range(2)]

                # ============ Phase 1: conv (25 taps, fp8 DoubleRow) ========
                with (
                    tc.tile_pool(name="xw", bufs=1) as xwp,
                    tc.tile_pool(name="cvps", bufs=1, space="PSUM") as cvps,
                ):
                    xpad_sb = xwp.tile([128, 2, HP, WP], dt.float8e4, tag="xp",
                                       name="xpad_sb")
                    nc.sync.dma_start(out=xpad_sb[:, :, 0:26, :],
                                      in_=xpad_d[:, :, 0:26, :])
                    nc.scalar.dma_start(out=xpad_sb[:, :, 26:HP, :],
                                        in_=xpad_d[:, :, 26:HP, :])
                    w25_sb = xwp.tile([128, 25, 2, 2, 128], dt.float8e4, tag="wt",
                                      name="w25_sb")
                    nc.gpsimd.dma_start(out=w25_sb, in_=w25_d[:])

                    for g, gt in enumerate(CONV_GROUPS):
                        P = [[cvps.tile([128, 480], dt.float32, tag=f"cv{m}{j}",
                                        name=f"P{g}{m}{j}", bufs=1)
                              for j in range(len(gt))] for m in range(2)]
                        for t in range(25):
                            dy, dx = offs[t]
                            for m in range(2):
                                lhsT = w25_sb[:, t, m]
                                for j, (p0, n, r0) in enumerate(gt):
                                    rows = n // W
                                    rhs = xpad_sb[:, :, r0 + 3 + dy: r0 + 3 + dy + rows,
                                                  3 + dx: 3 + dx + W]
                                    nc.tensor.matmul(P[m][j][:, 0:n], lhsT, rhs,
                                                     start=(t == 0), stop=(t == 24),
                                                     perf_mode=DR)
                        for m in range(2):
                            for j, (p0, n, r0) in enumerate(gt):
                                nc.vector.tensor_scalar_add(
                                    out=ms_hw[m][:, p0:p0 + n],
                                    in0=P[m][j][:, 0:n], scalar1=bsum_sb[m])

                ms3 = [ms_hw[k].rearrange("p (h w) -> p h w", w=W) for k in range(2)]

                # ======== Phase 2: projections + pair exchange ========
                with (
                    tc.tile_pool(name="pjps", bufs=1, space="PSUM") as pjps,
                    tc.tile_pool(name="pjcp", bufs=1) as pjcp,
                    tc.tile_pool(name="wproj", bufs=1) as wpp,
                ):
                    wqT_sb = [wpp.tile([128, CQ], dt.bfloat16, tag=f"wq{k}",
                                       name=f"wq{k}") for k in range(2)]
                    wkT_sb = [wpp.tile([128, CQ], dt.bfloat16, tag=f"wk{k}",
                                       name=f"wk{k}") for k in range(2)]
                    wvT_sb = [wpp.tile([128, 256], dt.bfloat16, tag=f"wv{k}",
                                       name=f"wv{k}") for k in range(2)]
                    for k in range(2):
                        nc.sync.dma_start(out=wkT_sb[k], in_=wkT_d[k])
                        nc.scalar.dma_start(out=wvT_sb[k], in_=wvT_d[k])
                        nc.sync.dma_start(out=wqT_sb[k], in_=wqT_d[k])

                    # k projection first so its exchange starts early
                    for n in range(NT_PROJ):
                        sl = slice(n * PROJ_N, (n + 1) * PROJ_N)
                        pk = pjps.tile([CQ, PROJ_N], dt.float32, tag="pq",
                                       name=f"pk{n}", bufs=2)
                        for k in range(2):
                            nc.tensor.matmul(pk, wkT_sb[k], ms_hw[k][:, sl],
                                             start=(k == 0), stop=(k == 1))
                        nc.vector.tensor_scalar_add(out=k_sb[:, sl], in0=pk,
                                                    scalar1=bk_sb)
                    nc.sync.dma_start(out=pack_k[:], in_=k_sb[:])
                    nc.gpsimd.collective_compute(
                        "AllGather", mybir.AluOpType.bypass,
                        replica_groups=[[0, 1], [2, 3], [4, 5], [6, 7]],
                        ins=[pack_k[:]], outs=[gath_k[:]])

                    # v^T (own half, w-major chunks) -> pack; two w-halves so
                    # the exchange overlaps the remaining compute
                    def vt_half(i):
                        for w0 in range(i * WH, (i + 1) * WH, 8):
                            stg8 = pjcp.tile([HC, 8, 256], dt.bfloat16, tag="stg",
                                             name=f"stg{w0}", bufs=3)
                            for j in range(8):
                                w = w0 + j
                                pvt = pjps.tile([HC, 256], dt.float32, tag="pvt",
                                                name=f"pvt{w}", bufs=4)
                                for k in range(2):
                                    nc.tensor.matmul(pvt, ms3[k][:, :, w], wvT_sb[k],
                                                     start=(k == 0), stop=(k == 1))
                                if w % 2 == 0:
                                    nc.vector.tensor_copy(out=stg8[:, j, :], in_=pvt)
                                else:
                                    nc.scalar.copy(out=stg8[:, j, :], in_=pvt)
                            nc.scalar.dma_start(
                                out=pack_v[i][w0 - i * WH:w0 - i * WH + 8]
                                .rearrange("w h c -> h w c"),
                                in_=stg8)
                        nc.gpsimd.collective_compute(
                            "AllGather", mybir.AluOpType.bypass,
                            replica_groups=[[0, 1], [2, 3], [4, 5], [6, 7]],
                            ins=[pack_v[i][:]], outs=[gath_v[i][:]])

                    vt_half(0)

                    # q projection (between the two v halves: unblocks energies)
                    for n in range(NT_PROJ):
                        sl = slice(n * PROJ_N, (n + 1) * PROJ_N)
                        pq = pjps.tile([CQ, PROJ_N], dt.float32, tag="pq",
                                       name=f"pq{n}", bufs=2)
                        for k in range(2):
                            nc.tensor.matmul(pq, wqT_sb[k], ms_hw[k][:, sl],
                                             start=(k == 0), stop=(k == 1))
                        nc.vector.tensor_scalar_add(out=q_sb[:, sl], in0=pq,
                                                    scalar1=bq_sb)

                    vt_half(1)

                    # own half back from pack_v in w-major layout for row-apply
                    nc.scalar.dma_start(out=v_whc[0:WH], in_=pack_v[0][:])
                    nc.sync.dma_start(out=v_whc[WH:W], in_=pack_v[1][:])
            # msp released here (frees ms before the big attention tensors)

            q3 = q_sb.rearrange("p (h w) -> p h w", w=W)
            k3 = k_sb.rearrange("p (h w) -> p h w", w=W)

            # ================= Phase 3: energies + batched exp =============
            with tc.tile_pool(name="gat", bufs=1) as gatp:
              with (
                tc.tile_pool(name="enps", bufs=1, space="PSUM") as enps,
                tc.tile_pool(name="encp", bufs=1) as encp,
              ):
                  # assemble full-H k and v^T from the gathered halves; the
                  # vT_wo re-layout DMAs are spread over 4 engine queues
                  k_full = gatp.tile([CQ, W, H], dt.bfloat16, tag="kf", name="k_full")
                  for gi in range(2):
                      for ci in range(2):
                          ko = encp.tile([CQ, NPOS // 2], dt.bfloat16, tag="ko",
                                         name=f"ko{gi}{ci}", bufs=2)
                          nc.sync.dma_start(
                              out=ko, in_=gath_k[gi][:, ci * (NPOS // 2):
                                                     (ci + 1) * (NPOS // 2)])
                          nc.vector.tensor_copy(
                              out=k_full[:, :, gi * HC + ci * (HC // 2):
                                          gi * HC + (ci + 1) * (HC // 2)],
                              in_=ko.rearrange("p (h w) -> p w h", w=W))
                  vT_wo = gatp.tile([H, W, 256], dt.bfloat16, tag="vt", name="vT_wo")
                  qs = [nc.sync, nc.scalar, nc.gpsimd, nc.sync]
                  for i in range(2):
                      for gi in range(2):
                          qs[2 * i + gi].dma_start(
                              out=vT_wo[gi * HC:(gi + 1) * HC,
                                        i * WH:(i + 1) * WH, :],
                              in_=gath_v[i][gi].rearrange("w h c -> h w c"))

                  # row (W) energies: only need own-half q/k; batch 4 h per bank
                  for h0 in range(0, HC, 4):
                      pew = enps.tile([W, 4, W], dt.float32, tag="ew",
                                      name=f"ew{h0}", bufs=3)
                      for j in range(4):
                          nc.tensor.matmul(pew[:, j, :], q3[:, h0 + j, :],
                                           k3[:, h0 + j, :], start=True, stop=True)
                      nc.scalar.activation(out=attW[:, h0:h0 + 4, :], in_=pew,
                                           func=mybir.ActivationFunctionType.Exp)
                  nc.vector.tensor_reduce(out=sW, in_=attW,
                                          axis=mybir.AxisListType.X,
                                          op=mybir.AluOpType.add)

                  # column (H) energies with diagonal mask; batch 4 w per bank;
                  # per-batch sH reduce pipelines with the matmuls
                  for w0 in range(0, W, 4):
                      peh = enps.tile([HC, 4, H], dt.float32, tag="eh",
                                      name=f"eh{w0}", bufs=3)
                      for j in range(4):
                          nc.tensor.matmul(peh[:, j, :], q3[:, :, w0 + j],
                                           k_full[:, w0 + j, :], start=True, stop=True)
                      nc.vector.tensor_add(
                          out=peh, in0=peh,
                          in1=dmask_sb[:, None, :].broadcast_to((HC, 4, H)))
                      nc.scalar.activation(out=attH[:, w0:w0 + 4, :], in_=peh,
                                           func=mybir.ActivationFunctionType.Exp)
                      nc.vector.tensor_reduce(out=sH[:, w0:w0 + 4],
                                              in_=attH[:, w0:w0 + 4, :],
                                              axis=mybir.AxisListType.X,
                                              op=mybir.AluOpType.add)

                  # joint softmax denominators
                  pt1 = enps.tile([HC, W], dt.float32, tag="tr", name="pt1", bufs=1)
                  nc.tensor.transpose(pt1, sW, id_f32)
                  nc.vector.tensor_add(out=s_h, in0=sH, in1=pt1)
                  nc.vector.reciprocal(out=recip_h, in_=s_h)
                  pt2 = enps.tile([W, HC], dt.float32, tag="tr2", name="pt2", bufs=1)
                  nc.tensor.transpose(pt2, recip_h, id_f32[0:HC, 0:HC])
                  nc.vector.tensor_copy(out=recip_w, in_=pt2)

                  # normalize att in place (batched; split vector/gpsimd)
                  for i, h0 in enumerate(range(0, HC, 4)):
                      eng = nc.vector if i % 2 == 0 else nc.gpsimd
                      eng.tensor_mul(
                          out=attW[:, h0:h0 + 4, :], in0=attW[:, h0:h0 + 4, :],
                          in1=recip_w[:, h0:h0 + 4][:, :, None].broadcast_to((W, 4, W)))
                  for i, w0 in enumerate(range(0, W, 4)):
                      eng = nc.vector if i % 2 == 0 else nc.gpsimd
                      eng.tensor_mul(
                          out=attH[:, w0:w0 + 4, :], in0=attH[:, w0:w0 + 4, :],
                          in1=recip_h[:, w0:w0 + 4][:, :, None].broadcast_to((HC, 4, H)))

              # ======== Phase 4a: row attention application ========
              xres0 = gatp.tile([128, NPOS], dt.float32, tag="xr0", name="xres0")
              nc.scalar.dma_start(out=xres0, in_=xres_d[0])
              with (
                  tc.tile_pool(name="apps", bufs=1, space="PSUM") as apps,
              ):
                  # transpose normalized att rows, batched PSUM drain
                  for h0 in range(0, HC, 8):
                      ptw = apps.tile([W, 8, W], dt.bfloat16, tag="tw",
                                      name=f"ptw{h0}", bufs=2)
                      for j in range(8):
                          nc.tensor.transpose(ptw[:, j, :], attW[:, h0 + j, :],
                                              id_bf[0:W, 0:W])
                      nc.vector.tensor_copy(out=awnT[:, h0:h0 + 8, :], in_=ptw)
                  for h0 in range(0, HC, 4):
                      po = [apps.tile([128, 4, W], dt.float32, tag=f"po{m}",
                                      name=f"po{h0}{m}", bufs=2) for m in range(2)]
                      for j in range(4):
                          for m in range(2):
                              nc.tensor.matmul(po[m][:, j, :],
                                               v_whc[:, h0 + j, m * 128:(m + 1) * 128],
                                               awnT[:, h0 + j, :],
                                               start=True, stop=True)
                      nc.vector.tensor_copy(
                          out=acc[0][:, (h0) * W:(h0 + 4) * W], in_=po[0])
                      nc.scalar.copy(
                          out=acc[1][:, (h0) * W:(h0 + 4) * W], in_=po[1])

              # ======== Phase 4b: column attention application ========
              acc3 = [acc[m].rearrange("p (h w) -> p h w", w=W) for m in range(2)]
              with (
                  tc.tile_pool(name="apps2", bufs=1, space="PSUM") as apps2,
              ):
                  for w0 in range(0, W, 8):
                      pth = apps2.tile([H, 8, HC], dt.bfloat16, tag="th",
                                       name=f"pth{w0}", bufs=2)
                      for j in range(8):
                          nc.tensor.transpose(pth[:, j, :], attH[:, w0 + j, :],
                                              id_bf[0:HC, 0:HC])
                      nc.vector.tensor_copy(out=ahnT[:, w0:w0 + 8, :], in_=pth)
                  for w0 in range(0, W, 8):
                      po2 = [apps2.tile([128, 8, HC], dt.float32, tag=f"po2{m}",
                                        name=f"po2{w0}{m}", bufs=2) for m in range(2)]
                      for j in range(8):
                          for m in range(2):
                              nc.tensor.matmul(po2[m][:, j, :],
                                               vT_wo[:, w0 + j, m * 128:(m + 1) * 128],
                                               ahnT[:, w0 + j, :],
                                               start=True, stop=True)
                      for m in range(2):
                          nc.vector.tensor_add(
                              out=acc3[m][:, :, w0:w0 + 8],
                              in0=acc3[m][:, :, w0:w0 + 8],
                              in1=po2[m].rearrange("p w h -> p h w"))

              # ======== Phase 5: residual + output ========
              with tc.tile_pool(name="fin", bufs=1) as finp:
                  for m in range(2):
                      for n in range(NT_PROJ):
                          sl = slice(n * PROJ_N, (n + 1) * PROJ_N)
                          if m == 0:
                              xr = xres0[:, sl]
                          else:
                              xr = finp.tile([128, PROJ_N], dt.float32, tag="xr",
                                             name=f"xr{m}{n}", bufs=4)
                              nc.scalar.dma_start(out=xr, in_=xres_d[m][:, sl])
                          fo = finp.tile([128, PROJ_N], dt.float32, tag="fo",
                                         name=f"fo{m}{n}", bufs=4)
                          nc.vector.scalar_tensor_tensor(
                              out=fo, in0=acc[m][:, sl], scalar=float(gamma_f),
                              in1=xr, op0=mybir.AluOpType.mult,
                              op1=mybir.AluOpType.add)
                          nc.sync.dma_start(out=out_d[m][:, sl], in_=fo)

    nc.compile()
    return nc


def _prepare_inputs(x, w_ms, b_ms, wq, bq, wk, bk, wv, bv, gamma):
    offs, taps = _fold_taps(np.asarray(w_ms, np.float32))
    x = np.asarray(x, np.float32)
    bsum = np.asarray(b_ms, np.float32).sum(0) * (XS * WS)
    gamma_f = float(np.asarray(gamma))
    bv = np.asarray(bv, np.float32)
    inv = 1.0 / (XS * WS)

    # fp8 folded conv weights, laid out [p, tap, m, k_hi, mc]
    w25 = np.empty((128, 25, 2, 2, 128), np.float32)
    for t, off in enumerate(offs):
        wt = taps[off].T.reshape(2, 128, 2, 128)     # [k, p, m, mc]
        w25[:, t] = wt.transpose(1, 2, 0, 3)         # -> [p, m, k, mc]
    w25 = (w25 * WS).astype(F8)
    wqT = (np.asarray(wq, np.float32).T * inv).reshape(2, 128, CQ).astype(BF16)
    wkT = (np.asarray(wk, np.float32).T * inv).reshape(2, 128, CQ).astype(BF16)
    wvT = (np.asarray(wv, np.float32).T * inv).reshape(2, 128, 256).astype(BF16)
    bq_a = np.ascontiguousarray(np.asarray(bq, np.float32).reshape(CQ, 1))
    bk_a = np.ascontiguousarray(np.asarray(bk, np.float32).reshape(CQ, 1))
    bsum_a = np.ascontiguousarray(bsum.reshape(2, 128, 1))

    in_maps = []
    for core in range(NCORES):
        b, g = core // 2, core % 2
        h0 = g * HC
        xp = np.zeros((C, H + 6, W + 6), np.float32)
        xp[:, 3:3 + H, 3:3 + W] = x[b]
        xpad = np.ascontiguousarray(
            (xp[:, h0:h0 + HP, :] * XS).reshape(2, 128, HP, WP)
            .transpose(1, 0, 2, 3)).astype(F8)
        dmask = np.zeros((HC, H), np.float32)
        dmask[np.arange(HC), h0 + np.arange(HC)] = NEG
        xres = (x[b, :, h0:h0 + HC, :].reshape(C, NPOS)
                + gamma_f * bv[:, None]).reshape(2, 128, NPOS)
        in_maps.append({
            "xpad": np.ascontiguousarray(xpad), "w25": w25, "wqT": wqT,
            "wkT": wkT, "wvT": wvT,
            "bq": bq_a, "bk": bk_a, "bsum": bsum_a, "dmask": dmask,
            "xres": np.ascontiguousarray(xres.astype(np.float32)),
        })
    return in_maps, gamma_f, offs


def run(inputs, trace=False):
    from concourse.bass_utils import run_bass_kernel_spmd
    in_maps, gamma_f, offs = _prepare_inputs(**inputs)
    nc = _build_program(gamma_f, offs)
    res = run_bass_kernel_spmd(nc, in_maps, list(range(NCORES)), trace=trace)
    out = np.empty((B, C, H, W), np.float32)
    for core in range(NCORES):
        b, g = core // 2, core % 2
        r = np.asarray(res.results[core]["out"]).reshape(C, HC, W)
        out[b, :, g * HC:(g + 1) * HC, :] = r
    return out, res


def kernel(**inputs) -> np.ndarray:
    out, _ = run(inputs, trace=False)
    return out
